# revision 1
# baseline (speedup 1.0000x reference)
"""GRU kernel for Trainium2, 8 NeuronCores, data-parallel over batch.

Reference semantics (per timestep t):
    xh    = concat(x_t, h)                 [B, D+H]
    z     = sigmoid(xh @ Wz.T + bz)        [B, H]
    r     = sigmoid(xh @ Wr.T + br)        [B, H]
    xrh   = concat(x_t, r * h)
    hcand = tanh(xrh @ Wc.T + bc)
    h     = (1 - z) * h + z * hcand
Output: hist [T, B, H] (h after every step).

Sharding: batch B=64 split 8 ways (8 rows/core), weights replicated.
No cross-core communication. Each core runs an identical program (SPMD).

On-chip layout ("packed T-layout"): a [B_l, H] tensor is stored as an
SBUF tile [128, 64] where partition p = h % 128 and free col = j*8 + b
with j = h // 128.  This makes the per-contract-tile moving operand of
every matmul a simple column slice, and keeps all elementwise ops on
identically-packed tiles.
"""

import numpy as np

T, B, D, H = 512, 64, 512, 1024
NCORES = 8
BL = B // NCORES          # 8 batch rows per core
NJ = H // 128             # 8 h tiles
ND = D // 128             # 4 d tiles
CHUNK = 16                # timesteps per x-chunk DMA
FCOL = NJ * BL            # 64 packed free columns

_cache = {}


def _build(t_steps):
    import concourse.bass as bass
    import concourse.tile as tile
    import concourse.mybir as mybir
    from concourse import bacc

    dt = mybir.dt.float32
    AF = mybir.ActivationFunctionType

    nc = bacc.Bacc(None, target_bir_lowering=False, debug=False)

    n_chunks = t_steps // CHUNK
    xc = nc.declare_dram_parameter("xc", [n_chunks, ND, 128, CHUNK, BL], dt,
                                   isOutput=False)
    h0T = nc.declare_dram_parameter("h0T", [128, FCOL], dt, isOutput=False)
    whT = nc.declare_dram_parameter("whT", [H, 3 * H], dt, isOutput=False)
    wxT = nc.declare_dram_parameter("wxT", [D, 3 * H], dt, isOutput=False)
    hist = nc.declare_dram_parameter("hist", [t_steps, 128, FCOL], dt,
                                     isOutput=True)

    with tile.TileContext(nc) as tc:
        with (
            tc.tile_pool(name="wpool", bufs=1) as wpool,
            tc.tile_pool(name="xpool", bufs=2) as xpool,
            tc.tile_pool(name="hpool", bufs=4) as hpool,
            tc.tile_pool(name="gpool", bufs=3) as gpool,
            tc.tile_pool(name="psum", bufs=3, space="PSUM") as psum_pool,
        ):
            # --- persistent weights ---
            wh = []
            for ch in range(NJ):
                wtile = wpool.tile([128, 3 * H], dt, tag=f"wh{ch}")
                nc.sync.dma_start(wtile[:], whT[ch * 128:(ch + 1) * 128, :])
                wh.append(wtile)
            wx = []
            for dtl in range(ND):
                wtile = wpool.tile([128, 3 * H], dt, tag=f"wx{dtl}")
                nc.sync.dma_start(wtile[:], wxT[dtl * 128:(dtl + 1) * 128, :])
                wx.append(wtile)

            h_prev = hpool.tile([128, FCOL], dt, tag="h")
            nc.sync.dma_start(h_prev[:], h0T[:])

            # psum packed regions (columns): z 0:64, r 64:128, c 128:192
            ZO, RO, CO = 0, FCOL, 2 * FCOL

            for c in range(n_chunks):
                xt = []
                for dtl in range(ND):
                    xtile = xpool.tile([128, CHUNK * BL], dt, tag=f"x{dtl}")
                    nc.sync.dma_start(xtile[:], xc[c, dtl])
                    xt.append(xtile)

                for it in range(CHUNK):
                    t = c * CHUNK + it
                    ps = psum_pool.tile([128, 3 * FCOL], dt, tag="ps")

                    def gate_mm(reg, gcol, moving):
                        # x part + h part accumulated per out-tile j
                        for j in range(NJ):
                            out = ps[:, reg + j * BL: reg + (j + 1) * BL]
                            for dtl in range(ND):
                                nc.tensor.matmul(
                                    out,
                                    wx[dtl][:, gcol + j * 128: gcol + (j + 1) * 128],
                                    xt[dtl][:, it * BL:(it + 1) * BL],
                                    start=(dtl == 0), stop=False,
                                )
                            for ch in range(NJ):
                                nc.tensor.matmul(
                                    out,
                                    wh[ch][:, gcol + j * 128: gcol + (j + 1) * 128],
                                    moving[:, ch * BL:(ch + 1) * BL],
                                    start=False, stop=(ch == NJ - 1),
                                )

                    # r gate first (critical path)
                    gate_mm(RO, H, h_prev)
                    rT = gpool.tile([128, FCOL], dt, tag="rT")
                    nc.scalar.activation(rT[:], ps[:, RO:RO + FCOL], AF.Sigmoid)
                    rhT = gpool.tile([128, FCOL], dt, tag="rhT")
                    nc.vector.tensor_mul(rhT[:], rT[:], h_prev[:])

                    # z gate (off critical path)
                    gate_mm(ZO, 0, h_prev)
                    zT = gpool.tile([128, FCOL], dt, tag="zT")
                    nc.scalar.activation(zT[:], ps[:, ZO:ZO + FCOL], AF.Sigmoid)

                    # candidate
                    gate_mm(CO, 2 * H, rhT)
                    hcT = gpool.tile([128, FCOL], dt, tag="hcT")
                    nc.scalar.activation(hcT[:], ps[:, CO:CO + FCOL], AF.Tanh)

                    # blend: h_new = h + z * (hc - h)
                    dT = gpool.tile([128, FCOL], dt, tag="dT")
                    nc.vector.tensor_sub(dT[:], hcT[:], h_prev[:])
                    eT = gpool.tile([128, FCOL], dt, tag="eT")
                    nc.vector.tensor_mul(eT[:], zT[:], dT[:])
                    h_new = hpool.tile([128, FCOL], dt, tag="h")
                    nc.vector.tensor_add(h_new[:], h_prev[:], eT[:])

                    nc.sync.dma_start(hist[t], h_new[:])
                    h_prev = h_new

    nc.compile()
    return nc


def _get_nc(t_steps):
    if t_steps not in _cache:
        _cache[t_steps] = _build(t_steps)
    return _cache[t_steps]


def _host_pack(x, h0, Wz, bz, Wr, br, Wc, bc, t_steps):
    n_chunks = t_steps // CHUNK
    whT = np.ascontiguousarray(
        np.concatenate([Wz[:, D:].T, Wr[:, D:].T, Wc[:, D:].T], axis=1))
    wxT = np.ascontiguousarray(
        np.concatenate([Wz[:, :D].T, Wr[:, :D].T, Wc[:, :D].T], axis=1))
    in_maps = []
    for k in range(NCORES):
        xl = x[:t_steps, k * BL:(k + 1) * BL, :]            # [T, 8, 512]
        xck = np.ascontiguousarray(
            xl.reshape(n_chunks, CHUNK, BL, ND, 128).transpose(0, 3, 4, 1, 2))
        h0l = h0[k * BL:(k + 1) * BL, :]                    # [8, 1024]
        h0Tk = np.ascontiguousarray(
            h0l.T.reshape(NJ, 128, BL).transpose(1, 0, 2).reshape(128, FCOL))
        in_maps.append({"xc": xck, "h0T": h0Tk, "whT": whT, "wxT": wxT})
    return in_maps


def _host_unpack(results, t_steps):
    outs = []
    for k in range(NCORES):
        hl = results[k]["hist"]                             # [T, 128, 64]
        hl = hl.reshape(t_steps, 128, NJ, BL).transpose(0, 3, 2, 1)
        outs.append(hl.reshape(t_steps, BL, H))
    return np.concatenate(outs, axis=1).astype(np.float32)  # [T, B, H]


def _run(x, h0, Wz, bz, Wr, br, Wc, bc, t_steps, trace=False):
    from concourse.bass_utils import run_bass_kernel_spmd
    assert not (np.any(bz) or np.any(br) or np.any(bc)), \
        "nonzero biases not supported by this kernel build"
    nc = _get_nc(t_steps)
    in_maps = _host_pack(x, h0, Wz, bz, Wr, br, Wc, bc, t_steps)
    res = run_bass_kernel_spmd(nc, in_maps, list(range(NCORES)), trace=trace)
    return _host_unpack(res.results, t_steps), res


def kernel(x, h0, Wz, bz, Wr, br, Wc, bc):
    out, _ = _run(np.asarray(x), np.asarray(h0), np.asarray(Wz),
                  np.asarray(bz), np.asarray(Wr), np.asarray(br),
                  np.asarray(Wc), np.asarray(bc), T)
    return out



# revision 2
# speedup vs baseline: 13.6355x; 13.6355x over previous
"""GRU kernel for Trainium2, 8 NeuronCores, data-parallel over batch.

Reference semantics (per timestep t):
    xh    = concat(x_t, h)                 [B, D+H]
    z     = sigmoid(xh @ Wz.T + bz)        [B, H]
    r     = sigmoid(xh @ Wr.T + br)        [B, H]
    xrh   = concat(x_t, r * h)
    hcand = tanh(xrh @ Wc.T + bc)
    h     = (1 - z) * h + z * hcand
Output: hist [T, B, H] (h after every step).

Sharding: batch B=64 split 8 ways (8 rows/core), weights replicated.
No cross-core communication; identical SPMD program on every core.

v2 design (vs fp32 baseline at 63.9 ms):
 - fp16 weights/activations for all matmuls (PSUM accumulates fp32).
   fp32 LDWEIGHTS is 2 instructions x 333 ns and fp32 matmul ~360 ns;
   fp16 gets 1 LDWEIGHTS with FastWeightLoad + 1-cycle/row matmul.
 - The x-contribution of every gate pre-activation is precomputed for
   all timesteps in one fat GEMM phase (moving dim 512), stored to a
   DRAM scratch buffer in per-step packed layout, and added to PSUM
   with one DVE op per gate in the loop. Cuts per-step tensor-engine
   pairs from 288 to 192.
 - Packed T-layout: [B_l, H] lives in SBUF as [128, 64] with partition
   p = h % 128 and free col = (h // 128) * 8 + b.
 - Per-gate PSUM tiles in separate banks; the candidate gate is split
   into two 32-col halves so its tanh/blend/cast tail overlaps the
   other half's matmuls and the next step's r-group.

h state kept in fp32 (blend in fp32); only matmul operands are fp16.
"""

import numpy as np

T, B, D, H = 512, 64, 512, 1024
NCORES = 8
BL = B // NCORES          # 8 batch rows per core
NJ = H // 128             # 8 h tiles
ND = D // 128             # 4 d tiles
FCOL = NJ * BL            # 64 packed free columns
HALF = FCOL // 2          # 32
PRE_T = 64                # timesteps per precompute chunk
PRE_N = PRE_T * BL        # 512 moving cols in precompute GEMM
CHUNK = 16                # timesteps per gx chunk in the loop

_cache = {}


def _build(t_steps):
    import concourse.tile as tile
    import concourse.mybir as mybir
    from concourse import bacc

    f32 = mybir.dt.float32
    f16 = mybir.dt.float16
    AF = mybir.ActivationFunctionType

    nc = bacc.Bacc(None, target_bir_lowering=False, debug=False)

    npre = t_steps // PRE_T
    nt16 = t_steps // CHUNK
    xc = nc.declare_dram_parameter("xc", [npre, ND, 128, PRE_N], f16,
                                   isOutput=False)
    h0T = nc.declare_dram_parameter("h0T", [128, FCOL], f32, isOutput=False)
    h0b = nc.declare_dram_parameter("h0b", [128, FCOL], f16, isOutput=False)
    whT = nc.declare_dram_parameter("whT", [H, 3 * H], f16, isOutput=False)
    wxT = nc.declare_dram_parameter("wxT", [D, 3 * H], f16, isOutput=False)
    hist = nc.declare_dram_parameter("hist", [t_steps, 2, 128, HALF], f32,
                                     isOutput=True)

    with tile.TileContext(nc) as tc:
        with (
            tc.tile_pool(name="wpool", bufs=1) as wpool,
            tc.tile_pool(name="gxdram", bufs=1, space="DRAM") as gxdram,
        ):
            wh = []
            for ch in range(NJ):
                whtile = wpool.tile([128, 3 * H], f16, tag=f"wh{ch}",
                                    name=f"wh{ch}")
                nc.sync.dma_start(whtile[:], whT[ch * 128:(ch + 1) * 128, :])
                wh.append(whtile)

            gxp = gxdram.tile([nt16, 3, 128, CHUNK * FCOL], f16, name="gxp")

            # ---------- Phase 1: gx[t] = x_t @ Wx.T for all t, 3 gates ----
            with (
                tc.tile_pool(name="wxpool", bufs=1) as wxpool,
                tc.tile_pool(name="pre_x", bufs=2) as pxp,
                tc.tile_pool(name="pre_s", bufs=2) as psp,
                tc.tile_pool(name="pre_ps", bufs=2, space="PSUM") as ppp,
            ):
                wx = []
                for d in range(ND):
                    wxtile = wxpool.tile([128, 3 * H], f16, tag=f"wx{d}",
                                         name=f"wx{d}")
                    nc.sync.dma_start(wxtile[:],
                                      wxT[d * 128:(d + 1) * 128, :])
                    wx.append(wxtile)

                nq = PRE_T // CHUNK    # 16-step subchunks per pre chunk
                for c in range(npre):
                    xt = []
                    for d in range(ND):
                        xtile = pxp.tile([128, PRE_N], f16, tag=f"x{d}",
                                         name=f"xt{d}")
                        nc.sync.dma_start(xtile[:], xc[c, d])
                        xt.append(xtile)
                    stg = {}
                    for q in range(nq):
                        for g in range(3):
                            s = psp.tile([128, CHUNK * FCOL], f16,
                                         tag=f"s{q}_{g}", name=f"stg{q}_{g}")
                            stg[(q, g)] = s
                    for g in range(3):
                        for j in range(NJ):
                            ps = ppp.tile([128, PRE_N], f32, tag="pps",
                                          name="pps")
                            for d in range(ND):
                                nc.tensor.matmul(
                                    ps[:],
                                    wx[d][:, g * H + j * 128:
                                          g * H + (j + 1) * 128],
                                    xt[d][:],
                                    start=(d == 0), stop=(d == ND - 1),
                                )
                            # scatter psum (cols = t*8+b) into per-step
                            # packed tiles (cols = u*64 + j*8 + b)
                            for q in range(nq):
                                src = ps[:, q * CHUNK * BL:
                                         (q + 1) * CHUNK * BL].rearrange(
                                    "p (u b) -> p u b", u=CHUNK)
                                dst = stg[(q, g)][:].rearrange(
                                    "p (u f) -> p u f", u=CHUNK)[
                                    :, :, j * BL:(j + 1) * BL]
                                nc.vector.tensor_copy(dst, src)
                    for q in range(nq):
                        for g in range(3):
                            nc.sync.dma_start(gxp[c * nq + q, g],
                                              stg[(q, g)][:])

            # ---------- Phase 2: the recurrent loop ----------
            with (
                tc.tile_pool(name="gxl", bufs=2) as gxl,
                tc.tile_pool(name="hp", bufs=3) as hp,
                tc.tile_pool(name="gp", bufs=2) as gp,
                tc.tile_pool(name="lps", bufs=2, space="PSUM") as lps,
            ):
                hA = hp.tile([128, HALF], f32, tag="hA", name="hA")
                hB = hp.tile([128, HALF], f32, tag="hB", name="hB")
                hbA = hp.tile([128, HALF], f16, tag="hbA", name="hbA")
                hbB = hp.tile([128, HALF], f16, tag="hbB", name="hbB")
                nc.sync.dma_start(hA[:], h0T[:, 0:HALF])
                nc.sync.dma_start(hB[:], h0T[:, HALF:FCOL])
                nc.sync.dma_start(hbA[:], h0b[:, 0:HALF])
                nc.sync.dma_start(hbB[:], h0b[:, HALF:FCOL])

                def hmm_group(ps, gcol, movA, movB, j0, j1):
                    # ch-outer so the first 32 matmuls only need movA;
                    # start=True only on the very first matmul (clears the
                    # whole bank's has_written bits; every element is
                    # first-touched by a ch==0 matmul which overwrites).
                    nmm = 0
                    last = (j1 - j0) * NJ
                    for ch in range(NJ):
                        mov = movA if ch < 4 else movB
                        msl = mov[:, (ch % 4) * BL:(ch % 4 + 1) * BL]
                        for j in range(j0, j1):
                            nmm += 1
                            nc.tensor.matmul(
                                ps[:, (j - j0) * BL:(j - j0 + 1) * BL],
                                wh[ch][:, gcol + j * 128:
                                       gcol + (j + 1) * 128],
                                msl,
                                start=(nmm == 1), stop=(nmm == last),
                            )

                for c16 in range(nt16):
                    gxt = []
                    for g in range(3):
                        gt = gxl.tile([128, CHUNK * FCOL], f16,
                                      tag=f"gx{g}", name=f"gxt{g}")
                        nc.sync.dma_start(gt[:], gxp[c16, g])
                        gxt.append(gt)
                    for u in range(CHUNK):
                        t = c16 * CHUNK + u
                        uc = u * FCOL
                        psr = lps.tile([128, FCOL], f32, tag="psr",
                                       name="psr")
                        psz = lps.tile([128, FCOL], f32, tag="psz",
                                       name="psz")
                        pscA = lps.tile([128, HALF], f32, tag="pscA",
                                        name="pscA")
                        pscB = lps.tile([128, HALF], f32, tag="pscB",
                                        name="pscB")

                        # r gate (critical path) then z (slack filler)
                        hmm_group(psr, H, hbA, hbB, 0, NJ)
                        nc.vector.tensor_add(psr[:], psr[:],
                                             gxt[1][:, uc:uc + FCOL])
                        rT = gp.tile([128, FCOL], f16, tag="rT", name="rT")
                        nc.scalar.activation(rT[:], psr[:], AF.Sigmoid)
                        rhA = gp.tile([128, HALF], f16, tag="rhA",
                                      name="rhA")
                        rhB = gp.tile([128, HALF], f16, tag="rhB",
                                      name="rhB")
                        nc.vector.tensor_mul(rhA[:], rT[:, 0:HALF], hbA[:])
                        nc.vector.tensor_mul(rhB[:], rT[:, HALF:FCOL],
                                             hbB[:])

                        hmm_group(psz, 0, hbA, hbB, 0, NJ)
                        nc.vector.tensor_add(psz[:], psz[:],
                                             gxt[0][:, uc:uc + FCOL])
                        zT = gp.tile([128, FCOL], f32, tag="zT", name="zT")
                        nc.scalar.activation(zT[:], psz[:], AF.Sigmoid)

                        # candidate gate in two halves
                        hmm_group(pscA, 2 * H, rhA, rhB, 0, NJ // 2)
                        hmm_group(pscB, 2 * H, rhA, rhB, NJ // 2, NJ)

                        hA_n = hp.tile([128, HALF], f32, tag="hA",
                                       name="hA")
                        hB_n = hp.tile([128, HALF], f32, tag="hB",
                                       name="hB")
                        hbA_n = hp.tile([128, HALF], f16, tag="hbA",
                                        name="hbA")
                        hbB_n = hp.tile([128, HALF], f16, tag="hbB",
                                        name="hbB")

                        halves = [(pscA, hA, hA_n, hbA_n, 0),
                                  (pscB, hB, hB_n, hbB_n, HALF)]
                        for hi, (psc, h_old, h_new, hb_new, c0) in \
                                enumerate(halves):
                            nc.vector.tensor_add(
                                psc[:], psc[:],
                                gxt[2][:, uc + c0:uc + c0 + HALF])
                            cT = gp.tile([128, HALF], f32, tag=f"cT{hi}",
                                         name=f"cT{hi}")
                            nc.scalar.activation(cT[:], psc[:], AF.Tanh)
                            dT = gp.tile([128, HALF], f32, tag=f"dT{hi}",
                                         name=f"dT{hi}")
                            nc.vector.tensor_sub(dT[:], cT[:], h_old[:])
                            eT = gp.tile([128, HALF], f32, tag=f"eT{hi}",
                                         name=f"eT{hi}")
                            nc.vector.tensor_mul(eT[:], zT[:, c0:c0 + HALF],
                                                 dT[:])
                            nc.vector.tensor_add(h_new[:], h_old[:], eT[:])
                            nc.scalar.activation(hb_new[:], h_new[:],
                                                 AF.Copy)
                            nc.sync.dma_start(hist[t, hi], h_new[:])

                        hA, hB, hbA, hbB = hA_n, hB_n, hbA_n, hbB_n

    nc.compile()
    return nc


def _get_nc(t_steps):
    if t_steps not in _cache:
        _cache[t_steps] = _build(t_steps)
    return _cache[t_steps]


def _host_pack(x, h0, Wz, bz, Wr, br, Wc, bc, t_steps):
    npre = t_steps // PRE_T
    whT = np.ascontiguousarray(
        np.concatenate([Wz[:, D:].T, Wr[:, D:].T, Wc[:, D:].T],
                       axis=1)).astype(np.float16)
    wxT = np.ascontiguousarray(
        np.concatenate([Wz[:, :D].T, Wr[:, :D].T, Wc[:, :D].T],
                       axis=1)).astype(np.float16)
    in_maps = []
    for k in range(NCORES):
        xl = x[:t_steps, k * BL:(k + 1) * BL, :]            # [T, 8, 512]
        xck = np.ascontiguousarray(
            xl.reshape(npre, PRE_T, BL, ND, 128)
              .transpose(0, 3, 4, 1, 2)
              .reshape(npre, ND, 128, PRE_N)).astype(np.float16)
        h0l = h0[k * BL:(k + 1) * BL, :]                    # [8, 1024]
        h0Tk = np.ascontiguousarray(
            h0l.T.reshape(NJ, 128, BL).transpose(1, 0, 2)
               .reshape(128, FCOL)).astype(np.float32)
        in_maps.append({"xc": xck, "h0T": h0Tk,
                        "h0b": h0Tk.astype(np.float16),
                        "whT": whT, "wxT": wxT})
    return in_maps


def _host_unpack(results, t_steps):
    outs = []
    for k in range(NCORES):
        hl = results[k]["hist"]                   # [T, 2, 128, 32]
        hl = hl.transpose(0, 2, 1, 3).reshape(t_steps, 128, FCOL)
        hl = hl.reshape(t_steps, 128, NJ, BL).transpose(0, 3, 2, 1)
        outs.append(hl.reshape(t_steps, BL, H))
    return np.concatenate(outs, axis=1).astype(np.float32)  # [T, B, H]


def _run(x, h0, Wz, bz, Wr, br, Wc, bc, t_steps, trace=False):
    from concourse.bass_utils import run_bass_kernel_spmd
    assert not (np.any(bz) or np.any(br) or np.any(bc)), \
        "nonzero biases not supported by this kernel build"
    nc = _get_nc(t_steps)
    in_maps = _host_pack(x, h0, Wz, bz, Wr, br, Wc, bc, t_steps)
    res = run_bass_kernel_spmd(nc, in_maps, list(range(NCORES)), trace=trace)
    return _host_unpack(res.results, t_steps), res


def kernel(x, h0, Wz, bz, Wr, br, Wc, bc):
    out, _ = _run(np.asarray(x), np.asarray(h0), np.asarray(Wz),
                  np.asarray(bz), np.asarray(Wr), np.asarray(br),
                  np.asarray(Wc), np.asarray(bc), T)
    return out


# revision 3
# speedup vs baseline: 15.5571x; 1.1409x over previous
"""GRU kernel for Trainium2, 8 NeuronCores, data-parallel over batch.

Reference semantics (per timestep t):
    xh    = concat(x_t, h)                 [B, D+H]
    z     = sigmoid(xh @ Wz.T + bz)        [B, H]
    r     = sigmoid(xh @ Wr.T + br)        [B, H]
    xrh   = concat(x_t, r * h)
    hcand = tanh(xrh @ Wc.T + bc)
    h     = (1 - z) * h + z * hcand
Output: hist [T, B, H] (h after every step).

Sharding: batch B=64 split 8 ways (8 rows/core), weights replicated.
No cross-core communication; identical SPMD program on every core.

v3 design (v2 measured 4.69 ms, PE 64% busy with two exposed serial
chains per step; fp32 baseline was 63.9 ms):
 - fp16 weights/operands for all matmuls (PSUM accumulates fp32);
   fp16 h state (no fp32 copy, no cast on the critical path).
 - x-contributions precomputed for all t in one fat GEMM phase
   (moving dim 512), stored to DRAM in per-step packed layout.
 - gx is injected into each gate's PSUM accumulation by an identity
   matmul (start=True), removing the DVE add from the critical chain.
 - Packed T-layout: [B_l, H] lives in SBUF as [128, 64] with partition
   p = h % 128 and free col = (h // 128) * 8 + b.
 - PSUM split: r and z gates in column halves, candidate in column
   quarters; h state in column quarters. The per-quarter
   tanh->sub->mul->add tail overlaps later quarters' matmuls and the
   next step's r-group, which consumes h quarter-by-quarter
   (ch-outer), so the PE almost never waits on the recurrence tail.
"""

import numpy as np

T, B, D, H = 512, 64, 512, 1024
NCORES = 8
BL = B // NCORES          # 8 batch rows per core
NJ = H // 128             # 8 h tiles
ND = D // 128             # 4 d tiles
FCOL = NJ * BL            # 64 packed free columns
HALF = FCOL // 2          # 32
QTR = FCOL // 4           # 16
PRE_T = 64                # timesteps per precompute chunk
PRE_N = PRE_T * BL        # 512 moving cols in precompute GEMM
CHUNK = 16                # timesteps per gx chunk in the loop

_cache = {}


def _build(t_steps):
    import concourse.tile as tile
    import concourse.mybir as mybir
    from concourse import bacc

    f32 = mybir.dt.float32
    f16 = mybir.dt.float16
    AF = mybir.ActivationFunctionType

    nc = bacc.Bacc(None, target_bir_lowering=False, debug=False)

    npre = t_steps // PRE_T
    nt16 = t_steps // CHUNK
    xc = nc.declare_dram_parameter("xc", [npre, ND, 128, PRE_N], f16,
                                   isOutput=False)
    h0b = nc.declare_dram_parameter("h0b", [128, FCOL], f16, isOutput=False)
    whT = nc.declare_dram_parameter("whT", [H, 3 * H], f16, isOutput=False)
    wxT = nc.declare_dram_parameter("wxT", [D, 3 * H], f16, isOutput=False)
    identD = nc.declare_dram_parameter("identD", [128, 128], f16,
                                       isOutput=False)
    hist = nc.declare_dram_parameter("hist", [t_steps, 4, 128, QTR], f16,
                                     isOutput=True)

    with tile.TileContext(nc) as tc:
        with (
            tc.tile_pool(name="wpool", bufs=1) as wpool,
            tc.tile_pool(name="gxdram", bufs=1, space="DRAM") as gxdram,
        ):
            wh = []
            for ch in range(NJ):
                whtile = wpool.tile([128, 3 * H], f16, tag=f"wh{ch}",
                                    name=f"wh{ch}")
                nc.sync.dma_start(whtile[:], whT[ch * 128:(ch + 1) * 128, :])
                wh.append(whtile)
            ident = wpool.tile([128, 128], f16, tag="ident", name="ident")
            nc.sync.dma_start(ident[:], identD[:])

            gxp = gxdram.tile([nt16, 3, 128, CHUNK * FCOL], f16, name="gxp")

            # ---------- Phase 1: gx[t] = x_t @ Wx.T for all t, 3 gates ----
            with (
                tc.tile_pool(name="wxpool", bufs=1) as wxpool,
                tc.tile_pool(name="pre_x", bufs=2) as pxp,
                tc.tile_pool(name="pre_s", bufs=2) as psp,
                tc.tile_pool(name="pre_ps", bufs=2, space="PSUM") as ppp,
            ):
                wx = []
                for d in range(ND):
                    wxtile = wxpool.tile([128, 3 * H], f16, tag=f"wx{d}",
                                         name=f"wx{d}")
                    nc.sync.dma_start(wxtile[:],
                                      wxT[d * 128:(d + 1) * 128, :])
                    wx.append(wxtile)

                nq = PRE_T // CHUNK    # 16-step subchunks per pre chunk
                for c in range(npre):
                    xt = []
                    for d in range(ND):
                        xtile = pxp.tile([128, PRE_N], f16, tag=f"x{d}",
                                         name=f"xt{d}")
                        nc.sync.dma_start(xtile[:], xc[c, d])
                        xt.append(xtile)
                    stg = {}
                    for q in range(nq):
                        for g in range(3):
                            s = psp.tile([128, CHUNK * FCOL], f16,
                                         tag=f"s{q}_{g}", name=f"stg{q}_{g}")
                            stg[(q, g)] = s
                    for g in range(3):
                        for j in range(NJ):
                            ps = ppp.tile([128, PRE_N], f32, tag="pps",
                                          name="pps")
                            for d in range(ND):
                                nc.tensor.matmul(
                                    ps[:],
                                    wx[d][:, g * H + j * 128:
                                          g * H + (j + 1) * 128],
                                    xt[d][:],
                                    start=(d == 0), stop=(d == ND - 1),
                                )
                            # scatter psum (cols = t*8+b) into per-step
                            # packed tiles (cols = u*64 + j*8 + b)
                            for q in range(nq):
                                src = ps[:, q * CHUNK * BL:
                                         (q + 1) * CHUNK * BL].rearrange(
                                    "p (u b) -> p u b", u=CHUNK)
                                dst = stg[(q, g)][:].rearrange(
                                    "p (u f) -> p u f", u=CHUNK)[
                                    :, :, j * BL:(j + 1) * BL]
                                nc.vector.tensor_copy(dst, src)
                    for q in range(nq):
                        for g in range(3):
                            nc.sync.dma_start(gxp[c * nq + q, g],
                                              stg[(q, g)][:])

            # ---------- Phase 2: the recurrent loop ----------
            with (
                tc.tile_pool(name="gxl", bufs=2) as gxl,
                tc.tile_pool(name="hp", bufs=3) as hp,
                tc.tile_pool(name="gp", bufs=2) as gp,
                tc.tile_pool(name="lps", bufs=1, space="PSUM") as lps,
            ):
                hq = []
                for q in range(4):
                    hqt = hp.tile([128, QTR], f16, tag=f"hq{q}",
                                  name=f"hq{q}")
                    nc.sync.dma_start(hqt[:], h0b[:, q * QTR:(q + 1) * QTR])
                    hq.append(hqt)

                for c16 in range(nt16):
                    gxt = []
                    for g in range(3):
                        gt = gxl.tile([128, CHUNK * FCOL], f16,
                                      tag=f"gx{g}", name=f"gxt{g}")
                        nc.sync.dma_start(gt[:], gxp[c16, g])
                        gxt.append(gt)
                    for u in range(CHUNK):
                        t = c16 * CHUNK + u
                        uc = u * FCOL

                        # --- r gate, halves A (j 0-3) / B (j 4-7) ---
                        rT = []
                        for hf in range(2):
                            psr = lps.tile([128, HALF], f32,
                                           tag=f"psr{hf}", name=f"psr{hf}")
                            nc.tensor.matmul(
                                psr[:], ident[:],
                                gxt[1][:, uc + hf * HALF:
                                       uc + (hf + 1) * HALF],
                                start=True, stop=False)
                            for ch in range(NJ):
                                msl = hq[ch // 2][:, (ch % 2) * BL:
                                                  (ch % 2 + 1) * BL]
                                for j in range(hf * 4, hf * 4 + 4):
                                    nc.tensor.matmul(
                                        psr[:, (j % 4) * BL:
                                            (j % 4 + 1) * BL],
                                        wh[ch][:, H + j * 128:
                                               H + (j + 1) * 128],
                                        msl, start=False,
                                        stop=(ch == NJ - 1
                                              and j == hf * 4 + 3))
                            rTh = gp.tile([128, HALF], f16, tag=f"rT{hf}",
                                          name=f"rT{hf}")
                            nc.scalar.activation(rTh[:], psr[:], AF.Sigmoid)
                            rT.append(rTh)
                        rh = []
                        for q in range(4):
                            rhq = gp.tile([128, QTR], f16, tag=f"rh{q}",
                                          name=f"rh{q}")
                            nc.vector.tensor_mul(
                                rhq[:],
                                rT[q // 2][:, (q % 2) * QTR:
                                           (q % 2 + 1) * QTR],
                                hq[q][:])
                            rh.append(rhq)

                        # --- z gate, halves ---
                        zT = []
                        for hf in range(2):
                            psz = lps.tile([128, HALF], f32,
                                           tag=f"psz{hf}", name=f"psz{hf}")
                            nc.tensor.matmul(
                                psz[:], ident[:],
                                gxt[0][:, uc + hf * HALF:
                                       uc + (hf + 1) * HALF],
                                start=True, stop=False)
                            for ch in range(NJ):
                                msl = hq[ch // 2][:, (ch % 2) * BL:
                                                  (ch % 2 + 1) * BL]
                                for j in range(hf * 4, hf * 4 + 4):
                                    nc.tensor.matmul(
                                        psz[:, (j % 4) * BL:
                                            (j % 4 + 1) * BL],
                                        wh[ch][:, j * 128:(j + 1) * 128],
                                        msl, start=False,
                                        stop=(ch == NJ - 1
                                              and j == hf * 4 + 3))
                            zTh = gp.tile([128, HALF], f32, tag=f"zT{hf}",
                                          name=f"zT{hf}")
                            nc.scalar.activation(zTh[:], psz[:], AF.Sigmoid)
                            zT.append(zTh)

                        # --- candidate gate, quarters ---
                        psc = []
                        for q in range(4):
                            pscq = lps.tile([128, QTR], f32,
                                            tag=f"psc{q}", name=f"psc{q}")
                            nc.tensor.matmul(
                                pscq[:], ident[:],
                                gxt[2][:, uc + q * QTR:
                                       uc + (q + 1) * QTR],
                                start=True, stop=False)
                            psc.append(pscq)
                        # c1: contraction chunks 0-3 (need rh quarters 0-1)
                        for ch in range(4):
                            msl = rh[ch // 2][:, (ch % 2) * BL:
                                              (ch % 2 + 1) * BL]
                            for j in range(NJ):
                                nc.tensor.matmul(
                                    psc[j // 2][:, (j % 2) * BL:
                                                (j % 2 + 1) * BL],
                                    wh[ch][:, 2 * H + j * 128:
                                           2 * H + (j + 1) * 128],
                                    msl, start=False, stop=False)
                        # c2: chunks 4-7, quarter-by-quarter completion;
                        # each finished quarter's tail overlaps the rest.
                        hq_new = [None] * 4
                        for q in range(4):
                            for ch in range(4, NJ):
                                msl = rh[ch // 2][:, (ch % 2) * BL:
                                                  (ch % 2 + 1) * BL]
                                for j in (2 * q, 2 * q + 1):
                                    nc.tensor.matmul(
                                        psc[q][:, (j % 2) * BL:
                                               (j % 2 + 1) * BL],
                                        wh[ch][:, 2 * H + j * 128:
                                               2 * H + (j + 1) * 128],
                                        msl, start=False,
                                        stop=(ch == NJ - 1
                                              and j == 2 * q + 1))
                            cQ = gp.tile([128, QTR], f32, tag=f"cQ{q}",
                                         name=f"cQ{q}")
                            nc.scalar.activation(cQ[:], psc[q][:], AF.Tanh)
                            dQ = gp.tile([128, QTR], f32, tag=f"dQ{q}",
                                         name=f"dQ{q}")
                            nc.vector.tensor_sub(dQ[:], cQ[:], hq[q][:])
                            nc.vector.tensor_mul(
                                dQ[:],
                                zT[q // 2][:, (q % 2) * QTR:
                                           (q % 2 + 1) * QTR],
                                dQ[:])
                            hqn = hp.tile([128, QTR], f16, tag=f"hq{q}",
                                          name=f"hq{q}")
                            nc.vector.tensor_add(hqn[:], hq[q][:], dQ[:])
                            nc.sync.dma_start(hist[t, q], hqn[:])
                            hq_new[q] = hqn

                        hq = hq_new

    nc.compile()
    return nc


def _get_nc(t_steps):
    if t_steps not in _cache:
        _cache[t_steps] = _build(t_steps)
    return _cache[t_steps]


def _host_pack(x, h0, Wz, bz, Wr, br, Wc, bc, t_steps):
    npre = t_steps // PRE_T
    whT = np.ascontiguousarray(
        np.concatenate([Wz[:, D:].T, Wr[:, D:].T, Wc[:, D:].T],
                       axis=1)).astype(np.float16)
    wxT = np.ascontiguousarray(
        np.concatenate([Wz[:, :D].T, Wr[:, :D].T, Wc[:, :D].T],
                       axis=1)).astype(np.float16)
    identD = np.eye(128, dtype=np.float16)
    in_maps = []
    for k in range(NCORES):
        xl = x[:t_steps, k * BL:(k + 1) * BL, :]            # [T, 8, 512]
        xck = np.ascontiguousarray(
            xl.reshape(npre, PRE_T, BL, ND, 128)
              .transpose(0, 3, 4, 1, 2)
              .reshape(npre, ND, 128, PRE_N)).astype(np.float16)
        h0l = h0[k * BL:(k + 1) * BL, :]                    # [8, 1024]
        h0b = np.ascontiguousarray(
            h0l.T.reshape(NJ, 128, BL).transpose(1, 0, 2)
               .reshape(128, FCOL)).astype(np.float16)
        in_maps.append({"xc": xck, "h0b": h0b,
                        "whT": whT, "wxT": wxT, "identD": identD})
    return in_maps


def _host_unpack(results, t_steps):
    outs = []
    for k in range(NCORES):
        hl = results[k]["hist"].astype(np.float32)  # [T, 4, 128, 16]
        hl = hl.transpose(0, 2, 1, 3).reshape(t_steps, 128, FCOL)
        hl = hl.reshape(t_steps, 128, NJ, BL).transpose(0, 3, 2, 1)
        outs.append(hl.reshape(t_steps, BL, H))
    return np.concatenate(outs, axis=1).astype(np.float32)  # [T, B, H]


def _run(x, h0, Wz, bz, Wr, br, Wc, bc, t_steps, trace=False):
    from concourse.bass_utils import run_bass_kernel_spmd
    assert not (np.any(bz) or np.any(br) or np.any(bc)), \
        "nonzero biases not supported by this kernel build"
    nc = _get_nc(t_steps)
    in_maps = _host_pack(x, h0, Wz, bz, Wr, br, Wc, bc, t_steps)
    res = run_bass_kernel_spmd(nc, in_maps, list(range(NCORES)), trace=trace)
    return _host_unpack(res.results, t_steps), res


def kernel(x, h0, Wz, bz, Wr, br, Wc, bc):
    out, _ = _run(np.asarray(x), np.asarray(h0), np.asarray(Wz),
                  np.asarray(bz), np.asarray(Wr), np.asarray(br),
                  np.asarray(Wc), np.asarray(bc), T)
    return out
